# revision 15
# baseline (speedup 1.0000x reference)
"""Trainium2 Bass kernel for nn_CoarsePyramid (nms_detection).

Data-parallel over batch: B=8 -> 8 NeuronCores, one batch element each.

Per-core pipeline (C=512, T=64, TF=256, CCONF=400, GROUPS=32):
  fm_short = CGR(feature, w_cur)            [512, 64]
  feat2    = CGR(feature, w_lr)             [1024, 64]   (also an output)
  prop_feature = boundary_pool(feat2, segments)          [1024, 64]
  prop_roi = CGR(boundary_pool(flf, frame_segments), w_roi)  [512, 64]
  out = CGR(cat(prop_roi, prop_feature, fm_short, conf), w_prop)  [512, 64]

conv1x1: PE matmuls in bf16 hi/lo split (x ~ xh+xl, w ~ wh+wl; psum +=
wh*xh + wh*xl + wl*xh, fp32 accumulate; ~1e-5 rel err) — ~4x faster than
native fp32 matmul on TRN2. Weights pre-transposed on host and shipped as
two bf16 tensors (same total bytes as fp32). Bias rides as an extra
contraction row (K=1 two-pass matmuls against a bf16 ones row; for
conv_prop it sits inside the K=17 conf tail tile).
GroupNorm: per-channel sum (DVE segmented reduce) + sum of squares (ACT
Square + DVE reduce) from PSUM, group-summed/broadcast via tiny fp32 PE
matmuls with 0/1 masks, applied fused with ReLU via ACT(Relu, scale, bias).
Boundary max pooling: full sparse max-table (levels 0..log2(Tin)) built by
DVE shifted-max in a position-major layout packed d-wide over channel
tiles; queries = 2 idempotent anchors per segment, gathered by GPSIMD
ap_gather (one call per half, 128 host-precomputed int16 indices); empty
segments zeroed via a broadcast 0/1 mask.
"""

import contextlib

import numpy as np
import ml_dtypes

import concourse.bass as bass
import concourse.bacc as bacc
import concourse.tile as tile
import concourse.mybir as mybir
from concourse import bass_utils

B, C, T, TF, CCONF = 8, 512, 64, 256, 400
GROUPS, EPS = 32, 1e-5
F32 = mybir.dt.float32
BF16 = mybir.dt.bfloat16
I16 = mybir.dt.int16
AF = mybir.ActivationFunctionType
ALU = mybir.AluOpType
AX = mybir.AxisListType

N_SEG = 64
NLEV2 = 7   # levels 0..6 for Tin=64
NLEVF = 9   # levels 0..8 for Tin=256
NANCH = 2
NT = 4      # output tiles per conv call (Cout=512 per call)

_COMPILED = {}
BF = ml_dtypes.bfloat16


# --------------------------------------------------------------------------
# host-side input prep
# --------------------------------------------------------------------------

def _hi_lo(a):
    hi = a.astype(BF)
    lo = (a - hi.astype(np.float32)).astype(BF)
    return hi, lo


def _wt_pad(w, b):
    """[Cout, Cin] weight + [Cout] bias -> hi/lo bf16 [Cin+1, Cout]."""
    wt = np.concatenate([w.T, b[None, :]], axis=0).astype(np.float32)
    hi, lo = _hi_lo(wt)
    return np.ascontiguousarray(hi), np.ascontiguousarray(lo)


def _gb(g, be):
    """gamma/beta [512] -> [128, 8]: cols [0:4] gamma tiles, [4:8] beta."""
    gt = g.reshape(NT, 128).T
    bt = be.reshape(NT, 128).T
    return np.concatenate([gt, bt], axis=1).astype(np.float32).copy()


def _pool_idx_mask(seg, tin, njj):
    """2-anchor sparse-table gather indices + empty mask.

    Returns idx [128, 16] int16 (two per-half wrapped blocks of
    NANCH*N_SEG indices into that half's [nlev*tin] table) and mask
    [1, 2*njj*N_SEG] bf16 (0 for empty segments), (half, jj)-major.
    """
    s = np.clip(np.floor(seg), 0, tin - 1).astype(np.int64)  # [N, 4]
    idx_h, msk = [], []
    for h in range(2):
        lo, hi = s[:, 2 * h], s[:, 2 * h + 1]
        ln = hi - lo + 1
        ok = ln >= 1
        ln_c = np.maximum(ln, 1)
        k = np.floor(np.log2(ln_c)).astype(np.int64)  # 2^k <= len
        step = 2 ** k
        a0 = k * tin + lo
        a1 = k * tin + np.maximum(hi - step + 1, 0)
        idx = np.stack([a0, a1], axis=0)              # [NANCH, N]
        idx_h.append(np.where(ok[None, :], idx, 0).reshape(-1))
        msk.extend([ok.astype(np.float32)] * njj)
    idx_flat = np.concatenate(idx_h)                  # [2*NANCH*N]
    wrapped = idx_flat.reshape(2, NANCH * N_SEG // 16, 16)
    wrapped = np.transpose(wrapped, (2, 0, 1)).reshape(16, -1)
    idx16 = np.tile(wrapped, (8, 1)).astype(np.int16).copy()  # [128, 16]
    mask = np.concatenate(msk)[None, :].astype(BF).copy()     # [1, 2*njj*N]
    return idx16, mask


def _host_prep(inputs):
    f = {k: np.asarray(v) for k, v in inputs.items()}
    shared = {}
    for nm in ("cur", "lr", "roi", "prop"):
        hi, lo = _wt_pad(f[f"w_{nm}"], f[f"b_{nm}"])
        shared[f"wth_{nm}"], shared[f"wtl_{nm}"] = hi, lo
    shared["gb_cur"] = _gb(f["g_cur"], f["be_cur"])
    shared["gb_lr_a"] = _gb(f["g_lr"][:512], f["be_lr"][:512])
    shared["gb_lr_b"] = _gb(f["g_lr"][512:], f["be_lr"][512:])
    shared["gb_roi"] = _gb(f["g_roi"], f["be_roi"])
    shared["gb_prop"] = _gb(f["g_prop"], f["be_prop"])
    p = np.arange(128)
    gm16 = (p[:, None] // 16 == np.arange(8)[None, :]).astype(np.float32)
    gm32 = (p[:, None] // 32 == np.arange(4)[None, :]).astype(np.float32)
    gmt = np.zeros((8, 256), np.float32)
    gmt[:, 0:128] = gm16.T
    gmt[0:4, 128:256] = gm32.T
    shared["gmt"] = gmt
    cf = np.concatenate(
        [shared.pop("gb_cur"), shared.pop("gb_lr_a"), shared.pop("gb_lr_b"),
         shared.pop("gb_roi"), shared.pop("gb_prop"), gm16, gm32], axis=1)

    in_maps = []
    for b in range(B):
        m = dict(shared)
        x0 = f["feature"][b].reshape(4, 128, T).transpose(1, 0, 2).reshape(128, -1)
        flf = f["frame_level_feature"][b].reshape(4, 128, TF)
        flf = flf.transpose(1, 0, 2).reshape(128, -1)
        confp = np.zeros((512, T), np.float32)
        confp[:CCONF] = f["conf_result_feature"][b]
        confp = confp.reshape(4, 128, T).transpose(1, 0, 2).reshape(128, -1)
        m["constf"] = np.concatenate(
            [cf, x0, flf, confp], axis=1).astype(np.float32).copy()
        idx2, em2 = _pool_idx_mask(f["segments"][b], T, 4)
        idxf, emf = _pool_idx_mask(f["frame_segments"][b], TF, 2)
        m["idxs"] = np.concatenate([idx2, idxf], axis=1).copy()
        cb = np.zeros((1, 1024), np.float32)
        cb[0, 0:128] = 1.0
        cb[0, 128:192] = 1.0   # ones64
        # 192:256 zeros64
        cb[0, 256:768] = em2[0].astype(np.float32)
        cb[0, 768:1024] = emf[0].astype(np.float32)
        m["constb"] = cb.astype(BF).copy()
        in_maps.append(m)
    return in_maps


# --------------------------------------------------------------------------
# device kernel
# --------------------------------------------------------------------------

def _conv_mm(tc, pools, wth, wtl, rhs_h, rhs_l, nkt, onesb, m0=0,
             last_k=None, kmap=None):
    """bf16 hi/lo conv1x1 (+bias) matmuls for NT=4 out tiles -> psum tile.

    wth/wtl: SBUF bf16 [128, nkt(+1), Cout_total]; rhs_h/rhs_l: per-k bf16
    [*, T] APs. If last_k is None, bias = K=1 two-pass matmul (wt tile nkt,
    partition 0) against onesb; else the final k-tile has K=last_k rows
    with the bias row included (rhs row last_k-1 is ones in rhs_h and
    zero in rhs_l).
    """
    nc = tc.nc
    ps = pools["psum"].tile([128, NT * T], F32, tag="conv_ps")
    if kmap is None:
        kmap = list(range(nkt))
    for m in range(NT):
        out_ap = ps[:, bass.ts(m, T)]
        for k in range(nkt):
            kw = kmap[k]
            kk = 128 if (last_k is None or kw < nkt - 1) else last_k
            wh = wth[0:kk, kw, bass.ts(m0 + m, 128)]
            wl = wtl[0:kk, kw, bass.ts(m0 + m, 128)]
            last = last_k is not None and k == nkt - 1
            nc.tensor.matmul(out_ap, wh, rhs_h[kw][0:kk, :],
                             start=(k == 0), stop=False)
            nc.tensor.matmul(out_ap, wh, rhs_l[kw][0:kk, :],
                             start=False, stop=False)
            nc.tensor.matmul(out_ap, wl, rhs_h[kw][0:kk, :],
                             start=False, stop=last)
        if last_k is None:
            nc.tensor.matmul(out_ap, wth[0:1, nkt, bass.ts(m0 + m, 128)],
                             onesb[0:1, :], start=False, stop=False)
            nc.tensor.matmul(out_ap, wtl[0:1, nkt, bass.ts(m0 + m, 128)],
                             onesb[0:1, :], start=False, stop=True)
    return ps


def _gn_relu(tc, pools, ps, gb, gmask, gmaskT, cnt, out_writes, zcol, epscol):
    """GroupNorm stats from the conv psum + fused scale/bias ReLU."""
    nc = tc.nc
    sb = pools["sbuf_small"]
    # per-channel sum + sum of squares
    s_ss = sb.tile([128, 2 * NT], F32, tag="s_ss")
    nc.vector.tensor_reduce(
        s_ss[:, 0:NT], ps[:].rearrange("p (m t) -> p m t", m=NT),
        axis=AX.X, op=ALU.add)
    sq = pools["scratch"].tile([128, NT * T], F32, tag="sq")
    nc.scalar.activation(sq[:], ps[:], AF.Square, bias=zcol[:, 0:1])
    nc.vector.tensor_reduce(
        s_ss[:, NT : 2 * NT], sq[:].rearrange("p (m t) -> p m t", m=NT),
        axis=AX.X, op=ALU.add)

    G = gmask.shape[-1]
    st_ps = pools["psum_small"].tile([G, 2 * NT], F32, tag="st_ps")
    nc.tensor.matmul(st_ps[:], gmask[:], s_ss[:], start=True, stop=True)
    mu_rs = sb.tile([G, 2 * NT], F32, tag="mu_rs")
    # mu = sum/cnt ; var = sumsq/cnt - mu^2 ; rs = 1/sqrt(var+eps)
    nc.vector.tensor_scalar_mul(mu_rs[:, 0:NT], st_ps[:, 0:NT], 1.0 / cnt)
    var = sb.tile([G, NT], F32, tag="var")
    nc.vector.tensor_scalar_mul(var[:], st_ps[:, NT:], 1.0 / cnt)
    mu2 = sb.tile([G, NT], F32, tag="mu2")
    nc.vector.tensor_tensor(mu2[:], mu_rs[:, 0:NT], mu_rs[:, 0:NT], ALU.mult)
    nc.vector.tensor_tensor(var[:], var[:], mu2[:], ALU.subtract)
    sd = sb.tile([G, NT], F32, tag="sd")
    nc.scalar.activation(sd[:], var[:], AF.Sqrt, bias=epscol[0:G, 0:1])
    nc.vector.reciprocal(mu_rs[:, NT:], sd[:])

    bc_ps = pools["psum_small"].tile([128, 2 * NT], F32, tag="bc_ps")
    nc.tensor.matmul(bc_ps[:], gmaskT[:], mu_rs[:], start=True, stop=True)
    mb = sb.tile([128, 2 * NT], F32, tag="mb")
    nc.scalar.copy(mb[:], bc_ps[:])
    # A = rs*gamma ; Bv = beta - mu*A
    a_b = sb.tile([128, 2 * NT], F32, tag="a_b")
    nc.vector.tensor_tensor(a_b[:, 0:NT], mb[:, NT:], gb[:, 0:NT], ALU.mult)
    tmp = sb.tile([128, NT], F32, tag="abtmp")
    nc.vector.tensor_tensor(tmp[:], mb[:, 0:NT], a_b[:, 0:NT], ALU.mult)
    nc.vector.tensor_tensor(a_b[:, NT:], gb[:, NT:], tmp[:], ALU.subtract)

    for m in range(NT):
        nc.scalar.activation(
            out_writes[m], ps[:, bass.ts(m, T)], AF.Relu,
            scale=a_b[:, m : m + 1], bias=a_b[:, NT + m : NT + m + 1])


def _build_tables(eng, tbl, tin, nlev):
    """Full sparse max-table on tbl [128, 2, nlev*tin, d] (level 0 = data).

    The position axis is packed d-wide over channel tiles; level k entry t
    = max(data[t .. t+2^k-1]); valid width tin - 2^k + 1, tails garbage.
    """
    v = tbl.rearrange("p h (l t) d -> p h l t d", l=nlev)
    for k in range(1, nlev):
        d1, w = 2 ** (k - 1), tin - 2 ** k + 1
        eng.tensor_tensor(
            v[:, :, k, 0:w, :],
            v[:, :, k - 1, 0:w, :],
            v[:, :, k - 1, d1 : d1 + w, :],
            ALU.max)


def _pool_query(tc, pools, tbl, idx, em, ones1b, njj, tin, nlev, out, nh=2):
    """2 anchors per proposal per half, d-packed gather, max, mask empties.

    tbl [128, nh, nlev*tin, njj]; out [128, nh, njj, N_SEG] (= [128, j, n]).
    """
    nc = tc.nc
    gout = pools["gout"].tile([128, nh, NANCH, N_SEG, njj], F32,
                              tag=f"gout{tin}")
    for h in range(nh):
        nc.gpsimd.ap_gather(
            gout[:, h, :, :, :],
            tbl[:, h, :, :],
            idx[:, h * 8 : (h + 1) * 8],
            channels=128, num_elems=nlev * tin, d=njj, num_idxs=NANCH * N_SEG)
    m1 = pools["scratch"].tile([128, nh, njj, N_SEG], F32, tag=f"pm1_{tin}")
    # max over the 2 anchors, transposing (n, jj) -> (jj, n)
    nc.vector.tensor_tensor(
        m1[:],
        gout[:, :, 0, :, :].rearrange("p h n j -> p h j n"),
        gout[:, :, 1, :, :].rearrange("p h n j -> p h j n"),
        ALU.max)
    # zero empty segments with the pre-broadcast 0/1 mask (em: [128, ...])
    nw = nh * njj * N_SEG
    nc.vector.tensor_tensor(
        out[:], m1[:],
        em[:, 0:nw].rearrange("p (h j n) -> p h j n", h=nh, j=njj),
        ALU.mult)


def _cast_hilo(nc, pools, src, name):
    """fp32 SBUF tensor -> (hi, lo) bf16 tensors of the same shape."""
    shp = list(src.shape)
    hi = pools["acts"].tile(shp, BF16, tag=name + "_h")
    lo = pools["acts"].tile(shp, BF16, tag=name + "_l")
    nc.scalar.copy(hi[:], src[:])
    nc.vector.tensor_tensor(lo[:], src[:], hi[:], ALU.subtract)
    return hi, lo


def _build_nc():
    nc = bacc.Bacc("TRN2", target_bir_lowering=False, debug=False,
                   enable_asserts=False, num_devices=B)

    din = {}
    def dram_in(name, shape, dtype=F32):
        din[name] = nc.dram_tensor(name, list(shape), dtype,
                                   kind="ExternalInput").ap()
        return din[name]

    NCF = 52 + 4 * T + 4 * TF + 4 * T
    dram_in("constf", (128, NCF))
    dram_in("constb", (1, 1024), BF16)
    dram_in("idxs", (128, 32), I16)
    dram_in("gmt", (8, 256))
    for nm, kr, co_ in [("cur", C + 1, C), ("lr", C + 1, 2 * C),
                        ("roi", C + 1, C), ("prop", 4 * C + CCONF + 1, C)]:
        dram_in(f"wth_{nm}", (kr, co_), BF16)
        dram_in(f"wtl_{nm}", (kr, co_), BF16)

    out_d = nc.dram_tensor("out", [C, T], F32, kind="ExternalOutput").ap()
    feat2_d = nc.dram_tensor("feat2", [2 * C, T], F32, kind="ExternalOutput").ap()

    with tile.TileContext(nc) as tc, contextlib.ExitStack() as ctx:
        pools = {
            "consts": ctx.enter_context(tc.tile_pool(name="consts", bufs=1)),
            "wts": ctx.enter_context(tc.tile_pool(name="wts", bufs=1)),
            "acts": ctx.enter_context(tc.tile_pool(name="acts", bufs=1)),
            "sbuf_small": ctx.enter_context(tc.tile_pool(name="sbs", bufs=2)),
            "scratch": ctx.enter_context(tc.tile_pool(name="scr", bufs=2)),
            "gout": ctx.enter_context(tc.tile_pool(name="gout", bufs=1)),
            "psum": ctx.enter_context(
                tc.tile_pool(name="psum", bufs=4, space="PSUM")),
            "psum_small": ctx.enter_context(
                tc.tile_pool(name="psums", bufs=1, space="PSUM")),
        }
        co, wp, ap_ = pools["consts"], pools["wts"], pools["acts"]

        def load(pool, name, shape, dtype=F32, src_ap=None):
            t = pool.tile(list(shape), dtype, tag=name)
            nc.sync.dma_start(t[:], src_ap if src_ap is not None else din[name][:])
            return t

        CONSTF = load(co, "constf", (128, 52 + 4 * T + 4 * TF + 4 * T))
        CONSTB = load(co, "constb", (1, 1024), BF16)
        IDXS = load(co, "idxs", (128, 32), I16)
        GMT = load(co, "gmt", (8, 256))

        # ---- weights (bf16 hi/lo, k-major tiles [128, nkt+1, Cout]);
        # queued right behind the const blob so convs can start early
        def load_wt(nm, nkt, cout, kr):
            ts_ = []
            for pre in ("wth", "wtl"):
                dram = din[f"{pre}_{nm}"]
                wt_t = wp.tile([128, nkt + 1, cout], BF16, tag=f"{pre}_{nm}")
                full = min(nkt + 1, (kr) // 128)
                nc.sync.dma_start(
                    wt_t[:, 0:full, :],
                    dram[0 : full * 128, :].rearrange("(k p) o -> p k o", p=128))
                rem = kr - full * 128
                if rem:
                    nc.sync.dma_start(wt_t[0:rem, full, :], dram[full * 128 :, :])
                ts_.append(wt_t)
            return ts_

        WTH_CUR, WTL_CUR = load_wt("cur", 4, C, C + 1)
        WTH_LR, WTL_LR = load_wt("lr", 4, 2 * C, C + 1)
        WTH_ROI, WTL_ROI = load_wt("roi", 4, C, C + 1)
        # wt_prop split so the non-prop_feature k-tiles land first
        pw = []
        for pre in ("wth", "wtl"):
            dram = din[f"{pre}_prop"]
            wt_t = wp.tile([128, 20, C], BF16, tag=f"{pre}_prop")
            nc.sync.dma_start(
                wt_t[:, 0:4, :],
                dram[0:512, :].rearrange("(k p) o -> p k o", p=128))
            nc.sync.dma_start(
                wt_t[:, 12:19, :],
                dram[1536 : 1536 + 7 * 128, :].rearrange("(k p) o -> p k o", p=128))
            nc.sync.dma_start(wt_t[0:17, 19, :], dram[2432:2449, :])
            nc.sync.dma_start(
                wt_t[:, 4:12, :],
                dram[512:1536, :].rearrange("(k p) o -> p k o", p=128))
            pw.append(wt_t)
        WTH_PROP, WTL_PROP = pw

        GB_CUR = CONSTF[:, 0:8]
        GB_LR_A = CONSTF[:, 8:16]
        GB_LR_B = CONSTF[:, 16:24]
        GB_ROI = CONSTF[:, 24:32]
        GB_PROP = CONSTF[:, 32:40]
        GM16 = CONSTF[:, 40:48]
        GM32 = CONSTF[:, 48:52]
        GM16T = GMT[:, 0:128]
        GM32T = GMT[0:4, 128:256]
        X0 = CONSTF[:, 52 : 52 + 4 * T].rearrange("p (j t) -> p j t", j=4)
        FLFRAW = CONSTF[:, 52 + 4 * T : 52 + 4 * T + 4 * TF].rearrange(
            "p (j t) -> p j t", j=4)
        CONF = CONSTF[:, 52 + 4 * T + 4 * TF :].rearrange("p (j t) -> p j t", j=4)
        ONES1B = CONSTB[:, 0:128]
        IDX2 = IDXS[:, 0:16]
        IDXF = IDXS[:, 16:32]
        EM2 = CONSTB[:, 256:768]
        EMF = CONSTB[:, 768:1024]

        zcol = co.tile([128, 1], F32, tag="zcol")
        nc.vector.memset(zcol[:], 0.0)
        epscol = co.tile([8, 1], F32, tag="epscol")
        nc.vector.memset(epscol[:], EPS)
        onesb = co.tile([1, T], BF16, tag="onesb")
        nc.vector.memset(onesb[:], 1.0)

        # broadcast the empty-segment masks to all partitions once, early
        # (depends only on inputs; keeps the PE stream unblocked later)
        mps = pools["psum_small"].tile([128, 8 * N_SEG], F32, tag="mps")
        nc.tensor.matmul(mps[:, 0:512], ONES1B[:], EM2[:], start=True, stop=True)
        MASK2 = co.tile([128, 512], F32, tag="mask2")
        nc.scalar.copy(MASK2[:], mps[:, 0:512])
        mpsf = pools["psum_small"].tile([128, 8 * N_SEG], F32, tag="mps")
        nc.tensor.matmul(mpsf[:, 0:256], ONES1B[:], EMF[:], start=True, stop=True)
        MASKF = co.tile([128, 256], F32, tag="maskf")
        nc.scalar.copy(MASKF[:], mpsf[:, 0:256])

        X0H, X0L = _cast_hilo(nc, pools, X0, "x0")

        # frame tables, position-major packed d=2 per half; built on GPSIMD
        # to keep the DVE free for conv GroupNorm stats
        TBLFP = ap_.tile([128, 2, NLEVF * TF, 2], F32, tag="tblfp")
        nc.vector.tensor_copy(
            TBLFP[:, :, 0:TF, :],
            FLFRAW[:].rearrange("p (h j) t -> p h t j", h=2))
        _build_tables(nc.vector, TBLFP, TF, NLEVF)
        PR = ap_.tile([128, 2, 2, T], F32, tag="pr")  # [128, (h jj)=j, n]
        _pool_query(tc, pools, TBLFP, IDXF, MASKF, ONES1B, 2, TF, NLEVF, PR)
        PRH, PRL = _cast_hilo(nc, pools, PR, "pr")

        CONFH, CONFL = _cast_hilo(nc, pools, CONF, "conf")
        # bias row for conv_prop: ones in hi, zeros in lo (partition 16)
        nc.sync.dma_start(CONFH[16:17, 3, :], din["constb"][0:1, 128:192])
        nc.sync.dma_start(CONFL[16:17, 3, :], din["constb"][0:1, 192:256])

        # ---- conv matmul phases back-to-back in the PE stream; each GN
        # chain overlaps the next conv's matmuls
        FMS = ap_.tile([128, 4, T], F32, tag="fms")
        FEAT2 = ap_.tile([128, 8, T], F32, tag="feat2")
        ROIC = ap_.tile([128, 4, T], F32, tag="roic")
        xh = [X0H[:, k, :] for k in range(4)]
        xl = [X0L[:, k, :] for k in range(4)]
        prh = [PRH[:, k // 2, k % 2, :] for k in range(4)]
        prl = [PRL[:, k // 2, k % 2, :] for k in range(4)]

        ps_cur = _conv_mm(tc, pools, WTH_CUR, WTL_CUR, xh, xl, 4, onesb)
        _gn_relu(tc, pools, ps_cur, GB_CUR, GM16, GM16T, 16 * T,
                 [FMS[:, m, :] for m in range(4)], zcol, epscol)
        ps_lra = _conv_mm(tc, pools, WTH_LR, WTL_LR, xh, xl, 4, onesb)
        _gn_relu(tc, pools, ps_lra, GB_LR_A, GM32, GM32T, 32 * T,
                 [FEAT2[:, m, :] for m in range(4)], zcol, epscol)
        ps_lrb = _conv_mm(tc, pools, WTH_LR, WTL_LR, xh, xl, 4, onesb, m0=4)
        _gn_relu(tc, pools, ps_lrb, GB_LR_B, GM32, GM32T, 32 * T,
                 [FEAT2[:, 4 + m, :] for m in range(4)], zcol, epscol)
        ps_roi = _conv_mm(tc, pools, WTH_ROI, WTL_ROI, prh, prl, 4, onesb)
        _gn_relu(tc, pools, ps_roi, GB_ROI, GM16, GM16T, 16 * T,
                 [ROIC[:, m, :] for m in range(4)], zcol, epscol)
        FMSH, FMSL = _cast_hilo(nc, pools, FMS, "fms")
        ROICH, ROICL = _cast_hilo(nc, pools, ROIC, "roic")
        nc.sync.dma_start(feat2_d.rearrange("(j p) t -> p j t", p=128), FEAT2[:])

        # ---- feat2 pooling (packed d=4 per half, per-half pipelines)
        TBL2P = ap_.tile([128, 2, NLEV2 * T, 4], F32, tag="tbl2p")
        PF = ap_.tile([128, 2, 4, T], F32, tag="pf")
        for h in range(2):
            nc.vector.tensor_copy(
                TBL2P[:, h, 0:T, :],
                FEAT2[:, 4 * h : 4 * h + 4, :].rearrange("p j t -> p t j"))
            _build_tables(nc.vector, TBL2P[:, h : h + 1, :, :], T, NLEV2)
            _pool_query(tc, pools, TBL2P[:, h : h + 1, :, :],
                        IDX2[:, 8 * h : 8 * h + 8],
                        MASK2[:, 256 * h : 256 * h + 256],
                        ONES1B, 4, T, NLEV2, PF[:, h : h + 1, :, :], nh=1)
        PFH, PFL = _cast_hilo(nc, pools, PF, "pf")

        # ---- conv_prop on the concat; PF-dependent k-tiles last (h0 then
        # h1), weight k-tile indices permuted to match
        korder = [0, 1, 2, 3, 12, 13, 14, 15, 16, 17, 18, 19, 4, 5, 6, 7,
                  8, 9, 10, 11]
        rhs_h = ([ROICH[:, k, :] for k in range(4)]
                 + [PFH[:, k // 4, k % 4, :] for k in range(8)]
                 + [FMSH[:, k, :] for k in range(4)]
                 + [CONFH[:, k, :] for k in range(4)])
        rhs_l = ([ROICL[:, k, :] for k in range(4)]
                 + [PFL[:, k // 4, k % 4, :] for k in range(8)]
                 + [FMSL[:, k, :] for k in range(4)]
                 + [CONFL[:, k, :] for k in range(4)])
        OUT = ap_.tile([128, 4, T], F32, tag="out_t")
        ps_prop = _conv_mm(tc, pools, WTH_PROP, WTL_PROP, rhs_h, rhs_l, 20,
                           onesb, last_k=17, kmap=korder)
        _gn_relu(tc, pools, ps_prop, GB_PROP, GM16, GM16T, 16 * T,
                 [OUT[:, m, :] for m in range(4)], zcol, epscol)
        nc.sync.dma_start(out_d.rearrange("(j p) t -> p j t", p=128), OUT[:])

    nc.compile()
    return nc


# --------------------------------------------------------------------------
# entry point
# --------------------------------------------------------------------------

def kernel(**inputs):
    if "nc" not in _COMPILED:
        _COMPILED["nc"] = _build_nc()
    nc = _COMPILED["nc"]
    in_maps = _host_prep(inputs)
    res = bass_utils.run_bass_kernel_spmd(nc, in_maps, core_ids=list(range(B)))
    outs = res.results
    out = np.stack([outs[b]["out"] for b in range(B)], axis=0)
    feat2 = np.stack([outs[b]["feat2"] for b in range(B)], axis=0)
    return out.astype(np.float32), feat2.astype(np.float32)


# revision 16
# speedup vs baseline: 1.0094x; 1.0094x over previous
"""Trainium2 Bass kernel for nn_CoarsePyramid (nms_detection).

Data-parallel over batch: B=8 -> 8 NeuronCores, one batch element each.

Per-core pipeline (C=512, T=64, TF=256, CCONF=400, GROUPS=32):
  fm_short = CGR(feature, w_cur)            [512, 64]
  feat2    = CGR(feature, w_lr)             [1024, 64]   (also an output)
  prop_feature = boundary_pool(feat2, segments)          [1024, 64]
  prop_roi = CGR(boundary_pool(flf, frame_segments), w_roi)  [512, 64]
  out = CGR(cat(prop_roi, prop_feature, fm_short, conf), w_prop)  [512, 64]

conv1x1: PE matmuls in bf16 hi/lo split (x ~ xh+xl, w ~ wh+wl; psum +=
wh*xh + wh*xl + wl*xh, fp32 accumulate; ~1e-5 rel err) — ~4x faster than
native fp32 matmul on TRN2. Weights pre-transposed on host and shipped as
two bf16 tensors (same total bytes as fp32). Bias rides as an extra
contraction row (K=1 two-pass matmuls against a bf16 ones row; for
conv_prop it sits inside the K=17 conf tail tile).
GroupNorm: per-channel sum (DVE segmented reduce) + sum of squares (ACT
Square + DVE reduce) from PSUM, group-summed/broadcast via tiny fp32 PE
matmuls with 0/1 masks, applied fused with ReLU via ACT(Relu, scale, bias).
Boundary max pooling: full sparse max-table (levels 0..log2(Tin)) built by
DVE shifted-max in a position-major layout packed d-wide over channel
tiles; queries = 2 idempotent anchors per segment, gathered by GPSIMD
ap_gather (one call per half, 128 host-precomputed int16 indices); empty
segments zeroed via a broadcast 0/1 mask.
"""

import contextlib

import numpy as np
import ml_dtypes

import concourse.bass as bass
import concourse.bacc as bacc
import concourse.tile as tile
import concourse.mybir as mybir
from concourse import bass_utils

B, C, T, TF, CCONF = 8, 512, 64, 256, 400
GROUPS, EPS = 32, 1e-5
F32 = mybir.dt.float32
BF16 = mybir.dt.bfloat16
I16 = mybir.dt.int16
AF = mybir.ActivationFunctionType
ALU = mybir.AluOpType
AX = mybir.AxisListType

N_SEG = 64
NLEV2 = 7   # levels 0..6 for Tin=64
NLEVF = 9   # levels 0..8 for Tin=256
NANCH = 2
NT = 4      # output tiles per conv call (Cout=512 per call)

_COMPILED = {}
BF = ml_dtypes.bfloat16


# --------------------------------------------------------------------------
# host-side input prep
# --------------------------------------------------------------------------

def _hi_lo(a):
    hi = a.astype(BF)
    lo = (a - hi.astype(np.float32)).astype(BF)
    return hi, lo


def _wt_pad(w, b):
    """[Cout, Cin] weight + [Cout] bias -> hi/lo bf16 [Cin+1, Cout]."""
    wt = np.concatenate([w.T, b[None, :]], axis=0).astype(np.float32)
    hi, lo = _hi_lo(wt)
    return np.ascontiguousarray(hi), np.ascontiguousarray(lo)


def _gb(g, be):
    """gamma/beta [512] -> [128, 8]: cols [0:4] gamma tiles, [4:8] beta."""
    gt = g.reshape(NT, 128).T
    bt = be.reshape(NT, 128).T
    return np.concatenate([gt, bt], axis=1).astype(np.float32).copy()


def _pool_idx_mask(seg, tin, njj):
    """2-anchor sparse-table gather indices + empty mask.

    Returns idx [128, 16] int16 (two per-half wrapped blocks of
    NANCH*N_SEG indices into that half's [nlev*tin] table) and mask
    [1, 2*njj*N_SEG] bf16 (0 for empty segments), (half, jj)-major.
    """
    s = np.clip(np.floor(seg), 0, tin - 1).astype(np.int64)  # [N, 4]
    idx_h, msk = [], []
    for h in range(2):
        lo, hi = s[:, 2 * h], s[:, 2 * h + 1]
        ln = hi - lo + 1
        ok = ln >= 1
        ln_c = np.maximum(ln, 1)
        k = np.floor(np.log2(ln_c)).astype(np.int64)  # 2^k <= len
        step = 2 ** k
        a0 = k * tin + lo
        a1 = k * tin + np.maximum(hi - step + 1, 0)
        idx = np.stack([a0, a1], axis=0)              # [NANCH, N]
        idx_h.append(np.where(ok[None, :], idx, 0).reshape(-1))
        msk.extend([ok.astype(np.float32)] * njj)
    idx_flat = np.concatenate(idx_h)                  # [2*NANCH*N]
    wrapped = idx_flat.reshape(2, NANCH * N_SEG // 16, 16)
    wrapped = np.transpose(wrapped, (2, 0, 1)).reshape(16, -1)
    idx16 = np.tile(wrapped, (8, 1)).astype(np.int16).copy()  # [128, 16]
    mask = np.concatenate(msk)[None, :].astype(BF).copy()     # [1, 2*njj*N]
    return idx16, mask


def _host_prep(inputs):
    f = {k: np.asarray(v) for k, v in inputs.items()}
    shared = {}
    for nm in ("cur", "lr", "roi", "prop"):
        hi, lo = _wt_pad(f[f"w_{nm}"], f[f"b_{nm}"])
        shared[f"wth_{nm}"], shared[f"wtl_{nm}"] = hi, lo
    shared["gb_cur"] = _gb(f["g_cur"], f["be_cur"])
    shared["gb_lr_a"] = _gb(f["g_lr"][:512], f["be_lr"][:512])
    shared["gb_lr_b"] = _gb(f["g_lr"][512:], f["be_lr"][512:])
    shared["gb_roi"] = _gb(f["g_roi"], f["be_roi"])
    shared["gb_prop"] = _gb(f["g_prop"], f["be_prop"])
    p = np.arange(128)
    gm16 = (p[:, None] // 16 == np.arange(8)[None, :]).astype(np.float32)
    gm32 = (p[:, None] // 32 == np.arange(4)[None, :]).astype(np.float32)
    gmt = np.zeros((8, 256), np.float32)
    gmt[:, 0:128] = gm16.T
    gmt[0:4, 128:256] = gm32.T
    shared["gmt"] = gmt
    cf = np.concatenate(
        [shared.pop("gb_cur"), shared.pop("gb_lr_a"), shared.pop("gb_lr_b"),
         shared.pop("gb_roi"), shared.pop("gb_prop"), gm16, gm32], axis=1)

    in_maps = []
    for b in range(B):
        m = dict(shared)
        x0 = f["feature"][b].reshape(4, 128, T).transpose(1, 0, 2).reshape(128, -1)
        flf = f["frame_level_feature"][b].reshape(4, 128, TF)
        flf = flf.transpose(1, 0, 2).reshape(128, -1)
        confp = np.zeros((512, T), np.float32)
        confp[:CCONF] = f["conf_result_feature"][b]
        confp = confp.reshape(4, 128, T).transpose(1, 0, 2).reshape(128, -1)
        m["constf"] = np.concatenate(
            [cf, x0, flf, confp], axis=1).astype(np.float32).copy()
        idx2, em2 = _pool_idx_mask(f["segments"][b], T, 4)
        idxf, emf = _pool_idx_mask(f["frame_segments"][b], TF, 2)
        m["idxs"] = np.concatenate([idx2, idxf], axis=1).copy()
        cb = np.zeros((1, 1024), np.float32)
        cb[0, 0:128] = 1.0
        cb[0, 128:192] = 1.0   # ones64
        # 192:256 zeros64
        cb[0, 256:768] = em2[0].astype(np.float32)
        cb[0, 768:1024] = emf[0].astype(np.float32)
        m["constb"] = cb.astype(BF).copy()
        in_maps.append(m)
    return in_maps


# --------------------------------------------------------------------------
# device kernel
# --------------------------------------------------------------------------

def _conv_mm(tc, pools, wth, wtl, rhs_h, rhs_l, nkt, onesb, m0=0,
             last_k=None, kmap=None):
    """bf16 hi/lo conv1x1 (+bias) matmuls for NT=4 out tiles -> psum tile.

    wth/wtl: SBUF bf16 [128, nkt(+1), Cout_total]; rhs_h/rhs_l: per-k bf16
    [*, T] APs. If last_k is None, bias = K=1 two-pass matmul (wt tile nkt,
    partition 0) against onesb; else the final k-tile has K=last_k rows
    with the bias row included (rhs row last_k-1 is ones in rhs_h and
    zero in rhs_l).
    """
    nc = tc.nc
    ps = pools["psum"].tile([128, NT * T], F32, tag="conv_ps")
    if kmap is None:
        kmap = list(range(nkt))
    for m in range(NT):
        out_ap = ps[:, bass.ts(m, T)]
        for k in range(nkt):
            kw = kmap[k]
            kk = 128 if (last_k is None or kw < nkt - 1) else last_k
            wh = wth[0:kk, kw, bass.ts(m0 + m, 128)]
            wl = wtl[0:kk, kw, bass.ts(m0 + m, 128)]
            last = last_k is not None and k == nkt - 1
            nc.tensor.matmul(out_ap, wh, rhs_h[kw][0:kk, :],
                             start=(k == 0), stop=False)
            nc.tensor.matmul(out_ap, wh, rhs_l[kw][0:kk, :],
                             start=False, stop=False)
            nc.tensor.matmul(out_ap, wl, rhs_h[kw][0:kk, :],
                             start=False, stop=last)
        if last_k is None:
            nc.tensor.matmul(out_ap, wth[0:1, nkt, bass.ts(m0 + m, 128)],
                             onesb[0:1, :], start=False, stop=False)
            nc.tensor.matmul(out_ap, wtl[0:1, nkt, bass.ts(m0 + m, 128)],
                             onesb[0:1, :], start=False, stop=True)
    return ps


def _gn_relu(tc, pools, ps, gb, gmask, gmaskT, cnt, out_writes, zcol, epscol):
    """GroupNorm stats from the conv psum + fused scale/bias ReLU."""
    nc = tc.nc
    sb = pools["sbuf_small"]
    # per-channel sum + sum of squares
    s_ss = sb.tile([128, 2 * NT], F32, tag="s_ss")
    nc.vector.tensor_reduce(
        s_ss[:, 0:NT], ps[:].rearrange("p (m t) -> p m t", m=NT),
        axis=AX.X, op=ALU.add)
    sq = pools["scratch"].tile([128, NT * T], F32, tag="sq")
    nc.scalar.activation(sq[:], ps[:], AF.Square, bias=zcol[:, 0:1])
    nc.vector.tensor_reduce(
        s_ss[:, NT : 2 * NT], sq[:].rearrange("p (m t) -> p m t", m=NT),
        axis=AX.X, op=ALU.add)

    G = gmask.shape[-1]
    st_ps = pools["psum_small"].tile([G, 2 * NT], F32, tag="st_ps")
    nc.tensor.matmul(st_ps[:], gmask[:], s_ss[:], start=True, stop=True)
    mu_rs = sb.tile([G, 2 * NT], F32, tag="mu_rs")
    # mu = sum/cnt ; var = sumsq/cnt - mu^2 ; rs = 1/sqrt(var+eps)
    nc.vector.tensor_scalar_mul(mu_rs[:, 0:NT], st_ps[:, 0:NT], 1.0 / cnt)
    var = sb.tile([G, NT], F32, tag="var")
    nc.vector.tensor_scalar_mul(var[:], st_ps[:, NT:], 1.0 / cnt)
    mu2 = sb.tile([G, NT], F32, tag="mu2")
    nc.vector.tensor_tensor(mu2[:], mu_rs[:, 0:NT], mu_rs[:, 0:NT], ALU.mult)
    nc.vector.tensor_tensor(var[:], var[:], mu2[:], ALU.subtract)
    sd = sb.tile([G, NT], F32, tag="sd")
    nc.scalar.activation(sd[:], var[:], AF.Sqrt, bias=epscol[0:G, 0:1])
    nc.vector.reciprocal(mu_rs[:, NT:], sd[:])

    bc_ps = pools["psum_small"].tile([128, 2 * NT], F32, tag="bc_ps")
    nc.tensor.matmul(bc_ps[:], gmaskT[:], mu_rs[:], start=True, stop=True)
    mb = sb.tile([128, 2 * NT], F32, tag="mb")
    nc.scalar.copy(mb[:], bc_ps[:])
    # A = rs*gamma ; Bv = beta - mu*A
    a_b = sb.tile([128, 2 * NT], F32, tag="a_b")
    nc.vector.tensor_tensor(a_b[:, 0:NT], mb[:, NT:], gb[:, 0:NT], ALU.mult)
    tmp = sb.tile([128, NT], F32, tag="abtmp")
    nc.vector.tensor_tensor(tmp[:], mb[:, 0:NT], a_b[:, 0:NT], ALU.mult)
    nc.vector.tensor_tensor(a_b[:, NT:], gb[:, NT:], tmp[:], ALU.subtract)

    for m in range(NT):
        nc.scalar.activation(
            out_writes[m], ps[:, bass.ts(m, T)], AF.Relu,
            scale=a_b[:, m : m + 1], bias=a_b[:, NT + m : NT + m + 1])


def _build_tables(eng, tbl, tin, nlev):
    """Full sparse max-table on tbl [128, 2, nlev*tin, d] (level 0 = data).

    The position axis is packed d-wide over channel tiles; level k entry t
    = max(data[t .. t+2^k-1]); valid width tin - 2^k + 1, tails garbage.
    """
    v = tbl.rearrange("p h (l t) d -> p h l t d", l=nlev)
    for k in range(1, nlev):
        d1, w = 2 ** (k - 1), tin - 2 ** k + 1
        eng.tensor_tensor(
            v[:, :, k, 0:w, :],
            v[:, :, k - 1, 0:w, :],
            v[:, :, k - 1, d1 : d1 + w, :],
            ALU.max)


def _pool_query(tc, pools, tbl, idx, em, ones1b, njj, tin, nlev, out, nh=2):
    """2 anchors per proposal per half, d-packed gather, max, mask empties.

    tbl [128, nh, nlev*tin, njj]; out [128, nh, njj, N_SEG] (= [128, j, n]).
    """
    nc = tc.nc
    gout = pools["gout"].tile([128, nh, NANCH, N_SEG, njj], F32,
                              tag=f"gout{tin}")
    for h in range(nh):
        nc.gpsimd.ap_gather(
            gout[:, h, :, :, :],
            tbl[:, h, :, :],
            idx[:, h * 8 : (h + 1) * 8],
            channels=128, num_elems=nlev * tin, d=njj, num_idxs=NANCH * N_SEG)
    m1 = pools["scratch"].tile([128, nh, njj, N_SEG], F32, tag=f"pm1_{tin}")
    # max over the 2 anchors, transposing (n, jj) -> (jj, n)
    nc.vector.tensor_tensor(
        m1[:],
        gout[:, :, 0, :, :].rearrange("p h n j -> p h j n"),
        gout[:, :, 1, :, :].rearrange("p h n j -> p h j n"),
        ALU.max)
    # zero empty segments with the pre-broadcast 0/1 mask (em: [128, ...])
    nw = nh * njj * N_SEG
    nc.vector.tensor_tensor(
        out[:], m1[:],
        em[:, 0:nw].rearrange("p (h j n) -> p h j n", h=nh, j=njj),
        ALU.mult)


def _cast_hilo(nc, pools, src, name):
    """fp32 SBUF tensor -> (hi, lo) bf16 tensors of the same shape."""
    shp = list(src.shape)
    hi = pools["acts"].tile(shp, BF16, tag=name + "_h")
    lo = pools["acts"].tile(shp, BF16, tag=name + "_l")
    nc.scalar.copy(hi[:], src[:])
    nc.vector.tensor_tensor(lo[:], src[:], hi[:], ALU.subtract)
    return hi, lo


def _build_nc():
    nc = bacc.Bacc("TRN2", target_bir_lowering=False, debug=False,
                   enable_asserts=False, num_devices=B)

    din = {}
    def dram_in(name, shape, dtype=F32):
        din[name] = nc.dram_tensor(name, list(shape), dtype,
                                   kind="ExternalInput").ap()
        return din[name]

    NCF = 52 + 4 * T + 4 * TF + 4 * T
    dram_in("constf", (128, NCF))
    dram_in("constb", (1, 1024), BF16)
    dram_in("idxs", (128, 32), I16)
    dram_in("gmt", (8, 256))
    for nm, kr, co_ in [("cur", C + 1, C), ("lr", C + 1, 2 * C),
                        ("roi", C + 1, C), ("prop", 4 * C + CCONF + 1, C)]:
        dram_in(f"wth_{nm}", (kr, co_), BF16)
        dram_in(f"wtl_{nm}", (kr, co_), BF16)

    out_d = nc.dram_tensor("out", [C, T], F32, kind="ExternalOutput").ap()
    feat2_d = nc.dram_tensor("feat2", [2 * C, T], F32, kind="ExternalOutput").ap()

    with tile.TileContext(nc) as tc, contextlib.ExitStack() as ctx:
        pools = {
            "consts": ctx.enter_context(tc.tile_pool(name="consts", bufs=1)),
            "wts": ctx.enter_context(tc.tile_pool(name="wts", bufs=1)),
            "acts": ctx.enter_context(tc.tile_pool(name="acts", bufs=1)),
            "sbuf_small": ctx.enter_context(tc.tile_pool(name="sbs", bufs=2)),
            "scratch": ctx.enter_context(tc.tile_pool(name="scr", bufs=2)),
            "gout": ctx.enter_context(tc.tile_pool(name="gout", bufs=2)),
            "psum": ctx.enter_context(
                tc.tile_pool(name="psum", bufs=4, space="PSUM")),
            "psum_small": ctx.enter_context(
                tc.tile_pool(name="psums", bufs=1, space="PSUM")),
        }
        co, wp, ap_ = pools["consts"], pools["wts"], pools["acts"]

        def load(pool, name, shape, dtype=F32, src_ap=None):
            t = pool.tile(list(shape), dtype, tag=name)
            nc.sync.dma_start(t[:], src_ap if src_ap is not None else din[name][:])
            return t

        CONSTF = load(co, "constf", (128, 52 + 4 * T + 4 * TF + 4 * T))

        # ---- weights (bf16 hi/lo, k-major tiles [128, nkt+1, Cout]);
        # queued right behind the const blob so convs can start early
        def load_wt(nm, nkt, cout, kr):
            ts_ = []
            for pre in ("wth", "wtl"):
                dram = din[f"{pre}_{nm}"]
                wt_t = wp.tile([128, nkt + 1, cout], BF16, tag=f"{pre}_{nm}")
                full = min(nkt + 1, (kr) // 128)
                nc.sync.dma_start(
                    wt_t[:, 0:full, :],
                    dram[0 : full * 128, :].rearrange("(k p) o -> p k o", p=128))
                rem = kr - full * 128
                if rem:
                    nc.sync.dma_start(wt_t[0:rem, full, :], dram[full * 128 :, :])
                ts_.append(wt_t)
            return ts_

        WTH_CUR, WTL_CUR = load_wt("cur", 4, C, C + 1)
        CONSTB = load(co, "constb", (1, 1024), BF16)
        IDXS = load(co, "idxs", (128, 32), I16)
        GMT = load(co, "gmt", (8, 256))
        WTH_LR, WTL_LR = load_wt("lr", 4, 2 * C, C + 1)
        WTH_ROI, WTL_ROI = load_wt("roi", 4, C, C + 1)
        # wt_prop split so the non-prop_feature k-tiles land first
        pw = []
        for pre in ("wth", "wtl"):
            dram = din[f"{pre}_prop"]
            wt_t = wp.tile([128, 20, C], BF16, tag=f"{pre}_prop")
            nc.sync.dma_start(
                wt_t[:, 0:4, :],
                dram[0:512, :].rearrange("(k p) o -> p k o", p=128))
            nc.sync.dma_start(
                wt_t[:, 12:19, :],
                dram[1536 : 1536 + 7 * 128, :].rearrange("(k p) o -> p k o", p=128))
            nc.sync.dma_start(wt_t[0:17, 19, :], dram[2432:2449, :])
            nc.sync.dma_start(
                wt_t[:, 4:12, :],
                dram[512:1536, :].rearrange("(k p) o -> p k o", p=128))
            pw.append(wt_t)
        WTH_PROP, WTL_PROP = pw

        GB_CUR = CONSTF[:, 0:8]
        GB_LR_A = CONSTF[:, 8:16]
        GB_LR_B = CONSTF[:, 16:24]
        GB_ROI = CONSTF[:, 24:32]
        GB_PROP = CONSTF[:, 32:40]
        GM16 = CONSTF[:, 40:48]
        GM32 = CONSTF[:, 48:52]
        GM16T = GMT[:, 0:128]
        GM32T = GMT[0:4, 128:256]
        X0 = CONSTF[:, 52 : 52 + 4 * T].rearrange("p (j t) -> p j t", j=4)
        FLFRAW = CONSTF[:, 52 + 4 * T : 52 + 4 * T + 4 * TF].rearrange(
            "p (j t) -> p j t", j=4)
        CONF = CONSTF[:, 52 + 4 * T + 4 * TF :].rearrange("p (j t) -> p j t", j=4)
        ONES1B = CONSTB[:, 0:128]
        IDX2 = IDXS[:, 0:16]
        IDXF = IDXS[:, 16:32]
        EM2 = CONSTB[:, 256:768]
        EMF = CONSTB[:, 768:1024]

        zcol = co.tile([128, 1], F32, tag="zcol")
        nc.vector.memset(zcol[:], 0.0)
        epscol = co.tile([8, 1], F32, tag="epscol")
        nc.vector.memset(epscol[:], EPS)
        onesb = co.tile([1, T], BF16, tag="onesb")
        nc.vector.memset(onesb[:], 1.0)

        # broadcast the empty-segment masks to all partitions once, early
        # (depends only on inputs; keeps the PE stream unblocked later)
        mps = pools["psum_small"].tile([128, 8 * N_SEG], F32, tag="mps")
        nc.tensor.matmul(mps[:, 0:512], ONES1B[:], EM2[:], start=True, stop=True)
        MASK2 = co.tile([128, 512], F32, tag="mask2")
        nc.scalar.copy(MASK2[:], mps[:, 0:512])
        mpsf = pools["psum_small"].tile([128, 8 * N_SEG], F32, tag="mps")
        nc.tensor.matmul(mpsf[:, 0:256], ONES1B[:], EMF[:], start=True, stop=True)
        MASKF = co.tile([128, 256], F32, tag="maskf")
        nc.scalar.copy(MASKF[:], mpsf[:, 0:256])

        X0H, X0L = _cast_hilo(nc, pools, X0, "x0")

        # frame tables, position-major packed d=2 per half; built on GPSIMD
        # to keep the DVE free for conv GroupNorm stats
        TBLFP = ap_.tile([128, 2, NLEVF * TF, 2], F32, tag="tblfp")
        nc.vector.tensor_copy(
            TBLFP[:, :, 0:TF, :],
            FLFRAW[:].rearrange("p (h j) t -> p h t j", h=2))
        _build_tables(nc.vector, TBLFP, TF, NLEVF)
        PR = ap_.tile([128, 2, 2, T], F32, tag="pr")  # [128, (h jj)=j, n]
        _pool_query(tc, pools, TBLFP, IDXF, MASKF, ONES1B, 2, TF, NLEVF, PR)
        PRH, PRL = _cast_hilo(nc, pools, PR, "pr")

        CONFH, CONFL = _cast_hilo(nc, pools, CONF, "conf")
        # bias row for conv_prop: ones in hi, zeros in lo (partition 16)
        nc.sync.dma_start(CONFH[16:17, 3, :], din["constb"][0:1, 128:192])
        nc.sync.dma_start(CONFL[16:17, 3, :], din["constb"][0:1, 192:256])

        # ---- conv matmul phases back-to-back in the PE stream; each GN
        # chain overlaps the next conv's matmuls
        FMS = ap_.tile([128, 4, T], F32, tag="fms")
        FEAT2 = ap_.tile([128, 8, T], F32, tag="feat2")
        ROIC = ap_.tile([128, 4, T], F32, tag="roic")
        xh = [X0H[:, k, :] for k in range(4)]
        xl = [X0L[:, k, :] for k in range(4)]
        prh = [PRH[:, k // 2, k % 2, :] for k in range(4)]
        prl = [PRL[:, k // 2, k % 2, :] for k in range(4)]

        ps_cur = _conv_mm(tc, pools, WTH_CUR, WTL_CUR, xh, xl, 4, onesb)
        _gn_relu(tc, pools, ps_cur, GB_CUR, GM16, GM16T, 16 * T,
                 [FMS[:, m, :] for m in range(4)], zcol, epscol)
        ps_lra = _conv_mm(tc, pools, WTH_LR, WTL_LR, xh, xl, 4, onesb)
        _gn_relu(tc, pools, ps_lra, GB_LR_A, GM32, GM32T, 32 * T,
                 [FEAT2[:, m, :] for m in range(4)], zcol, epscol)
        ps_lrb = _conv_mm(tc, pools, WTH_LR, WTL_LR, xh, xl, 4, onesb, m0=4)
        _gn_relu(tc, pools, ps_lrb, GB_LR_B, GM32, GM32T, 32 * T,
                 [FEAT2[:, 4 + m, :] for m in range(4)], zcol, epscol)
        FMSH, FMSL = _cast_hilo(nc, pools, FMS, "fms")

        # feat2 pooling per half, interleaved with the remaining GN work so
        # each half's table/gather chain starts as soon as its half of
        # FEAT2 exists
        TBL2P = ap_.tile([128, 2, NLEV2 * T, 4], F32, tag="tbl2p")
        PF = ap_.tile([128, 2, 4, T], F32, tag="pf")
        PFH = pools["acts"].tile([128, 2, 4, T], BF16, tag="pf_h")
        PFL = pools["acts"].tile([128, 2, 4, T], BF16, tag="pf_l")

        def feat2_half(h):
            nc.vector.tensor_copy(
                TBL2P[:, h, 0:T, :],
                FEAT2[:, 4 * h : 4 * h + 4, :].rearrange("p j t -> p t j"))
            _build_tables(nc.vector, TBL2P[:, h : h + 1, :, :], T, NLEV2)
            _pool_query(tc, pools, TBL2P[:, h : h + 1, :, :],
                        IDX2[:, 8 * h : 8 * h + 8],
                        MASK2[:, 256 * h : 256 * h + 256],
                        ONES1B, 4, T, NLEV2, PF[:, h : h + 1, :, :], nh=1)
            nc.scalar.copy(PFH[:, h, :, :], PF[:, h, :, :])
            nc.vector.tensor_tensor(PFL[:, h, :, :], PF[:, h, :, :],
                                    PFH[:, h, :, :], ALU.subtract)

        feat2_half(0)
        ps_roi = _conv_mm(tc, pools, WTH_ROI, WTL_ROI, prh, prl, 4, onesb)
        _gn_relu(tc, pools, ps_roi, GB_ROI, GM16, GM16T, 16 * T,
                 [ROIC[:, m, :] for m in range(4)], zcol, epscol)
        ROICH, ROICL = _cast_hilo(nc, pools, ROIC, "roic")
        feat2_half(1)
        nc.sync.dma_start(feat2_d.rearrange("(j p) t -> p j t", p=128), FEAT2[:])

        # ---- conv_prop on the concat; PF-dependent k-tiles last (h0 then
        # h1), weight k-tile indices permuted to match
        korder = [0, 1, 2, 3, 12, 13, 14, 15, 16, 17, 18, 19, 4, 5, 6, 7,
                  8, 9, 10, 11]
        rhs_h = ([ROICH[:, k, :] for k in range(4)]
                 + [PFH[:, k // 4, k % 4, :] for k in range(8)]
                 + [FMSH[:, k, :] for k in range(4)]
                 + [CONFH[:, k, :] for k in range(4)])
        rhs_l = ([ROICL[:, k, :] for k in range(4)]
                 + [PFL[:, k // 4, k % 4, :] for k in range(8)]
                 + [FMSL[:, k, :] for k in range(4)]
                 + [CONFL[:, k, :] for k in range(4)])
        OUT = ap_.tile([128, 4, T], F32, tag="out_t")
        ps_prop = _conv_mm(tc, pools, WTH_PROP, WTL_PROP, rhs_h, rhs_l, 20,
                           onesb, last_k=17, kmap=korder)
        _gn_relu(tc, pools, ps_prop, GB_PROP, GM16, GM16T, 16 * T,
                 [OUT[:, m, :] for m in range(4)], zcol, epscol)
        nc.sync.dma_start(out_d.rearrange("(j p) t -> p j t", p=128), OUT[:])

    nc.compile()
    return nc


# --------------------------------------------------------------------------
# entry point
# --------------------------------------------------------------------------

def kernel(**inputs):
    if "nc" not in _COMPILED:
        _COMPILED["nc"] = _build_nc()
    nc = _COMPILED["nc"]
    in_maps = _host_prep(inputs)
    res = bass_utils.run_bass_kernel_spmd(nc, in_maps, core_ids=list(range(B)))
    outs = res.results
    out = np.stack([outs[b]["out"] for b in range(B)], axis=0)
    feat2 = np.stack([outs[b]["feat2"] for b in range(B)], axis=0)
    return out.astype(np.float32), feat2.astype(np.float32)
